# revision 1
# baseline (speedup 1.0000x reference)
"""BertSelfAttention (relative_key + skim-mask softmax) Trainium2 kernel.

Sharding: 8 cores = 4 batches x 2 head-halves. Each core handles one batch
and 8 heads (columns h*64..h*64+63 of Q/K/V for its head-half).

Device pipeline per core:
  1. QKV projections (bf16 matmuls, hidden pre-transposed host-side).
  2. Per head: windowed qd[l, j] = q[l,:] @ dist_embT[:, j] matmuls; the
     PSUM->SBUF copy is followed by a DMA whose *DRAM-side* access pattern
     absorbs the per-row diagonal shift (row stride 1280, per-partition
     extra +1), so DRAM holds qdcR[l, y] = qd[l, l+1151-y].
  3. XBAR transpose-DMA reloads [r, l]-oriented relative-position bias tiles
     directly: bias[r, l] = qdcR[l, 128+r] = qd[l, l-r+1023].
  4. bias -> PSUM via identity matmul; K^T Q accumulates on top; ACT computes
     exps = exp(scores + per-partition mask bias) in bf16.
  5. PV matmul with ones-column on V gives ctx^T and softmax denominators.
  6. Small PE transposes + per-partition reciprocal -> normalized output.
"""

import os
import sys

sys.path.insert(0, "/opt/trn_rl_repo")

import numpy as np
import ml_dtypes

import concourse.bass as bass
import concourse.tile as tile
from concourse import bacc, mybir
from concourse.bass_utils import run_bass_kernel_spmd

B, S, HID, H, D = 4, 1024, 1024, 16, 64
MAXP = 1024
EPS = 1e-8
NEG = -30.0          # additive bias for masked columns (exp -> ~1e-13)
HPC = 8              # heads per core
ODC = HPC * D        # 512 output dims per core
NJ = 2048            # reversed dist table columns
WIN = 1152           # qd j-window per 128-row l-chunk
RSTRIDE = 1280       # qdr DRAM row stride (>= WIN + 127 + 1)
SCALE = 1.0 / 8.0    # 1/sqrt(D)

BF16 = mybir.dt.bfloat16
F32 = mybir.dt.float32
NPBF16 = ml_dtypes.bfloat16

EXPF = mybir.ActivationFunctionType.Exp


def _body(nc, tc, s):
    """One full forward pass; s = dict of persistent tiles/handles."""
    dzlo_sb, dzhi_sb, mb_sb, id_sb = (
        s["dzlo_sb"], s["dzhi_sb"], s["mb_sb"], s["id_sb"],
    )
    QTb, KI, QB, Vb, out_sb, qdr = (
        s["QTb"], s["KI"], s["QB"], s["Vb"], s["out_sb"], s["qdr"],
    )

    # ---------------- stage 1: projections ----------------
    with (
        tc.tile_pool(name="stg1", bufs=1) as stg1,
        tc.tile_pool(name="projp", bufs=4, space="PSUM") as projp,
    ):
        hT_sb = stg1.tile([128, HID // 128, S], BF16)
        nc.sync.dma_start(
            hT_sb[:], s["hT"].ap().rearrange("(k p) s -> p k s", p=128)
        )
        wq_sb = stg1.tile([128, HID // 128, ODC], BF16)
        nc.sync.dma_start(
            wq_sb[:], s["wq"].ap().rearrange("(k p) o -> p k o", p=128)
        )
        wk_sb = stg1.tile([128, HID // 128, ODC], BF16)
        nc.sync.dma_start(
            wk_sb[:], s["wk"].ap().rearrange("(k p) o -> p k o", p=128)
        )
        wv_sb = stg1.tile([128, HID // 128, ODC], BF16)
        nc.sync.dma_start(
            wv_sb[:], s["wv"].ap().rearrange("(k p) o -> p k o", p=128)
        )
        for m in range(4):
            for sc in range(2):
                ps = projp.tile([128, 512], F32, tag="proj", name="psq")
                for k in range(8):
                    nc.tensor.matmul(
                        ps[:],
                        lhsT=wq_sb[:, k, m * 128 : (m + 1) * 128],
                        rhs=hT_sb[:, k, sc * 512 : (sc + 1) * 512],
                        start=(k == 0),
                        stop=(k == 7),
                    )
                nc.vector.tensor_scalar_mul(
                    QTb[:, m, sc * 512 : (sc + 1) * 512], ps[:], SCALE
                )
        for m in range(4):
            # QB top/bottom halves: Q^T rows for even/odd head (both slots)
            for slot in range(2):
                nc.gpsimd.tensor_copy(
                    out=QB[2 * m][0:64, slot, :], in_=QTb[0:64, m, :]
                )
                nc.gpsimd.tensor_copy(
                    out=QB[2 * m + 1][64:128, slot, :], in_=QTb[64:128, m, :]
                )
        for m in range(4):
            for sc in range(2):
                ps = projp.tile([128, 512], F32, tag="proj", name="psk")
                for k in range(8):
                    nc.tensor.matmul(
                        ps[:],
                        lhsT=wk_sb[:, k, m * 128 : (m + 1) * 128],
                        rhs=hT_sb[:, k, sc * 512 : (sc + 1) * 512],
                        start=(k == 0),
                        stop=(k == 7),
                    )
                ss = slice(sc * 512, (sc + 1) * 512)
                nc.any.tensor_copy(out=KI[2 * m][0:64, ss], in_=ps[0:64, :])
                nc.any.tensor_copy(
                    out=KI[2 * m + 1][64:128, ss], in_=ps[64:128, :]
                )
        for sc in range(8):
            ps = projp.tile([128, 512], F32, tag="proj", name="psv")
            for k in range(8):
                nc.tensor.matmul(
                    ps[:],
                    lhsT=hT_sb[:, k, sc * 128 : (sc + 1) * 128],
                    rhs=wv_sb[:, k, :],
                    start=(k == 0),
                    stop=(k == 7),
                )
            nc.any.tensor_copy(
                out=Vb[:, sc, :, 0:D],
                in_=ps[:].rearrange("p (h dd) -> p h dd", dd=D),
            )

    # ---------------- stages 2-4: per head-pair ----------------
    copy_engines = [nc.vector, nc.vector, nc.scalar, nc.vector, nc.scalar]
    cctr = [0]

    def pcopy(out, in_):
        eng = copy_engines[cctr[0] % len(copy_engines)]
        cctr[0] += 1
        if eng is nc.scalar:
            nc.scalar.activation(out, in_, mybir.ActivationFunctionType.Copy)
        else:
            nc.vector.tensor_copy(out=out, in_=in_)

    with (
        tc.tile_pool(name="qdp", bufs=2, space="PSUM") as qdp,
        tc.tile_pool(name="scoresp", bufs=3, space="PSUM") as scoresp,
        tc.tile_pool(name="ctxp", bufs=2, space="PSUM") as ctxp,
        tc.tile_pool(name="trp", bufs=1, space="PSUM") as trp,
        tc.tile_pool(name="qdsbp", bufs=3) as qdsbp,
        tc.tile_pool(name="expsp", bufs=2) as expsp,
        tc.tile_pool(name="ctxsbp", bufs=3) as ctxsbp,
        tc.tile_pool(name="smallp", bufs=8) as smallp,
    ):
        for hp in range(4):
            heads = [2 * hp, 2 * hp + 1]

            # stage 2: qd tiles + skewed DRAM writes. All matmuls K=128:
            # lhsT is the full head-pair Q^T slice; the half-zeroed dist
            # tables select one head's contribution.
            for L in range(8):
                AL = 896 - 128 * L
                qd_sbs = [
                    qdsbp.tile([128, WIN], BF16, tag=f"qd_sb{p}", name=f"qd_sb{p}")
                    for p in range(2)
                ]
                qslice = QTb[:, hp, L * 128 : (L + 1) * 128]
                for n0, nn in ((0, 512), (512, 512), (1024, 128)):
                    pss = []
                    for phB in range(2):
                        dz = dzlo_sb if phB == 0 else dzhi_sb
                        ps = qdp.tile([128, 512], F32, tag="qdps", name="qdps")
                        nc.tensor.matmul(
                            ps[:, :nn],
                            lhsT=qslice,
                            rhs=dz[:, AL + n0 : AL + n0 + nn],
                            start=True,
                            stop=True,
                        )
                        pss.append(ps)
                    for phB in range(2):
                        pcopy(qd_sbs[phB][:, n0 : n0 + nn], pss[phB][:, :nn])
                for phB in range(2):
                    dst = bass.AP(
                        tensor=qdr[heads[phB]],
                        offset=L * 128 * RSTRIDE,
                        ap=[[RSTRIDE + 1, 128], [1, WIN]],
                    )
                    nc.sync.dma_start(dst, qd_sbs[phB][:])

            # stage 3: scores^T = K^T Q + bias via single K=128 matmuls with
            # combined operands: lhsT = [K^T ; tiled I64] (KI), rhs =
            # [Q^T ; bias half] (QB). Odd heads use the flipped row layout.
            exps = [
                expsp.tile([128, 8, S], BF16, tag=f"exps{phB}", name=f"exps{phB}")
                for phB in range(2)
            ]
            for R in range(8):
                for lc in range(2):
                    ls = slice(lc * 512, (lc + 1) * 512)
                    if lc == 0:
                        # load this R's bias halves into QB slots 0 (lo), 1 (hi)
                        for phB in range(2):
                            h = heads[phB]
                            for half in range(2):
                                src = bass.AP(
                                    tensor=qdr[h],
                                    offset=128 + R * 128 + 64 * half,
                                    ap=[[RSTRIDE, S], [1, 64]],
                                )
                                dstp = slice(64, 128) if phB == 0 else slice(0, 64)
                                nc.sync.dma_start_transpose(
                                    QB[h][dstp, half, :], src
                                )
                    sc = [
                        scoresp.tile([128, 512], F32, tag="scores", name="sc_ps")
                        for _ in range(2)
                    ]
                    for half in range(2):
                        for phB in range(2):
                            h = heads[phB]
                            rs = slice(
                                R * 128 + 64 * half, R * 128 + 64 * half + 64
                            )
                            nc.tensor.matmul(
                                sc[phB][64 * half : 64 * half + 64, :],
                                lhsT=KI[h][:, rs],
                                rhs=QB[h][:, half, ls],
                                start=True,
                                stop=True,
                                tile_position=(0, 64 * half),
                            )
                    for phB in range(2):
                        nc.scalar.activation(
                            exps[phB][:, R, ls],
                            sc[phB][:],
                            EXPF,
                            bias=mb_sb[:, R : R + 1],
                        )

            # stage 4: PV + denominators + normalize + output
            for phB in range(2):
                h = heads[phB]
                for lc in range(2):
                    ct_ps = ctxp.tile([128, 512], F32, tag="ctx", name="ct_ps")
                    for R in range(8):
                        nc.tensor.matmul(
                            ct_ps[0 : D + 1, :],
                            lhsT=Vb[:, R, h, :],
                            rhs=exps[phB][:, R, lc * 512 : (lc + 1) * 512],
                            start=(R == 0),
                            stop=(R == 7),
                        )
                    ctx_sb = ctxsbp.tile(
                        [D + 1, 512], BF16, tag="ctx_sb", name="ctx_sb"
                    )
                    nc.any.tensor_copy(out=ctx_sb[:], in_=ct_ps[0 : D + 1, :])
                    for q in range(4):
                        tr_ps = trp.tile([128, D + 1], BF16, tag="tr", name="tr_ps")
                        nc.tensor.transpose(
                            tr_ps[:],
                            ctx_sb[:, q * 128 : (q + 1) * 128],
                            id_sb[0 : D + 1, 0 : D + 1],
                        )
                        den = smallp.tile([128, 1], F32, tag="den", name="den")
                        nc.vector.tensor_scalar_add(
                            den[:], tr_ps[:, D : D + 1], EPS
                        )
                        rec = smallp.tile([128, 1], F32, tag="rec", name="rec")
                        nc.vector.reciprocal(rec[:], den[:])
                        c = lc * 4 + q
                        nc.vector.tensor_scalar_mul(
                            out_sb[:, c, h, :], tr_ps[:, 0:D], rec[:]
                        )


def build_program(n_reps=1):
    nc = bacc.Bacc(trn_type="TRN2", target_bir_lowering=False, debug=False)

    hT = nc.dram_tensor("hT", [HID, S], BF16, kind="ExternalInput")
    wq = nc.dram_tensor("wq", [HID, ODC], BF16, kind="ExternalInput")
    wk = nc.dram_tensor("wk", [HID, ODC], BF16, kind="ExternalInput")
    wv = nc.dram_tensor("wv", [HID, ODC], BF16, kind="ExternalInput")
    distZlo = nc.dram_tensor("distZlo", [128, NJ], BF16, kind="ExternalInput")
    distZhi = nc.dram_tensor("distZhi", [128, NJ], BF16, kind="ExternalInput")
    irep = nc.dram_tensor("irep", [64, S], BF16, kind="ExternalInput")
    mbias = nc.dram_tensor("mbias", [128, 8], F32, kind="ExternalInput")
    ident = nc.dram_tensor("ident", [128, 128], BF16, kind="ExternalInput")
    out = nc.dram_tensor("out", [S, ODC], F32, kind="ExternalOutput")

    # per-head DRAM scratch for the skew-compacted qd rows
    qdr = [nc.dram_tensor(f"qdr{h}", [S * RSTRIDE], BF16) for h in range(HPC)]

    with tile.TileContext(nc) as tc:
        with tc.tile_pool(name="singles", bufs=1) as singles:
            dzlo_sb = singles.tile([128, NJ], BF16)
            nc.sync.dma_start(dzlo_sb[:], distZlo.ap())
            dzhi_sb = singles.tile([128, NJ], BF16)
            nc.sync.dma_start(dzhi_sb[:], distZhi.ap())
            mb_sb = singles.tile([128, 8], F32)
            nc.sync.dma_start(mb_sb[:], mbias.ap())
            id_sb = singles.tile([128, 128], BF16)
            nc.sync.dma_start(id_sb[:], ident.ap())

            QTb = singles.tile([128, 4, S], BF16)   # [od%128, od//128, s], x1/8
            # KI[h]: combined scores lhsT: even h: rows 0-63 K^T, 64-127 I64
            # tiled; odd h: flipped.
            KI = [
                singles.tile([128, S], BF16, name=f"KI{h}") for h in range(HPC)
            ]
            # QB[h]: combined scores rhs: even h: rows 0-63 Q^T, 64-127 bias
            # half (slot 0 = r-lo, slot 1 = r-hi); odd h: flipped.
            QB = [
                singles.tile([128, 2, S], BF16, name=f"QB{h}") for h in range(HPC)
            ]
            for h in range(HPC):
                if h % 2 == 0:
                    nc.sync.dma_start(KI[h][64:128, :], irep.ap())
                else:
                    nc.sync.dma_start(KI[h][0:64, :], irep.ap())
            # V natural with ones column: [s%128, s//128, h, 65]
            Vb = singles.tile([128, 8, HPC, D + 1], BF16)
            out_sb = singles.tile([128, 8, HPC, D], F32)

            nc.vector.memset(Vb[:, :, :, D : D + 1], 1.0)

            state = dict(
                hT=hT, wq=wq, wk=wk, wv=wv,
                dzlo_sb=dzlo_sb, dzhi_sb=dzhi_sb, mb_sb=mb_sb, id_sb=id_sb,
                QTb=QTb, KI=KI, QB=QB, Vb=Vb, out_sb=out_sb, qdr=qdr,
            )
            for _rep in range(n_reps):
                _body(nc, tc, state)

            nc.sync.dma_start(
                out.ap().rearrange("(c p) (h d) -> p c h d", p=128, d=D), out_sb[:]
            )

    nc.compile()
    return nc


def make_core_inputs(hidden_states, attention_mask, skim_mask, Wq, Wk, Wv, dist_emb):
    """Host-side prep: returns list of 8 in_maps."""
    hidden_states = np.asarray(hidden_states, np.float32)
    attention_mask = np.asarray(attention_mask, np.float32)
    skim_mask = np.asarray(skim_mask)
    Wq = np.asarray(Wq, np.float32)
    Wk = np.asarray(Wk, np.float32)
    Wv = np.asarray(Wv, np.float32)
    dist_emb = np.asarray(dist_emb, np.float32)

    # reversed dist tables: dist[d, xg] = dist_emb[2047 - xg, d], col 0 = 0;
    # "lo" has rows 0-63 active (even heads), "hi" rows 64-127 (odd heads).
    dzlo = np.zeros((128, NJ), np.float32)
    tmp = dist_emb[::-1].T  # [64, 2047]; tmp[d, i] = dist_emb[2046 - i, d]
    dzlo[0:64, 1:NJ] = tmp
    dzhi = np.zeros((128, NJ), np.float32)
    dzhi[64:128, 1:NJ] = tmp
    dzlo = np.ascontiguousarray(dzlo.astype(NPBF16))
    dzhi = np.ascontiguousarray(dzhi.astype(NPBF16))

    ident = np.ascontiguousarray(np.eye(128, dtype=NPBF16))
    irep = np.ascontiguousarray(np.tile(np.eye(64, dtype=NPBF16), (1, S // 64)))

    in_maps = []
    for core in range(8):
        b, hh = core // 2, core % 2
        cols = slice(hh * ODC, (hh + 1) * ODC)
        hT = np.ascontiguousarray(hidden_states[b].T.astype(NPBF16))
        mb = (
            attention_mask[b, 0, 0, :] + NEG * (1.0 - skim_mask[b].astype(np.float32))
        ).astype(np.float32)
        in_maps.append(
            {
                "hT": hT,
                "wq": np.ascontiguousarray(Wq[:, cols].astype(NPBF16)),
                "wk": np.ascontiguousarray(Wk[:, cols].astype(NPBF16)),
                "wv": np.ascontiguousarray(Wv[:, cols].astype(NPBF16)),
                "distZlo": dzlo,
                "distZhi": dzhi,
                "irep": irep,
                "mbias": np.ascontiguousarray(mb.reshape(8, 128).T),
                "ident": ident,
            }
        )
    return in_maps


def kernel(
    hidden_states,
    attention_mask,
    skim_mask,
    Wq,
    bq,
    Wk,
    bk,
    Wv,
    bv,
    dist_emb,
):
    in_maps = make_core_inputs(
        hidden_states, attention_mask, skim_mask, Wq, Wk, Wv, dist_emb
    )
    nc = build_program()
    res = run_bass_kernel_spmd(nc, in_maps, core_ids=list(range(8)))
    out = np.zeros((B, S, HID), np.float32)
    for core in range(8):
        b, hh = core // 2, core % 2
        out[b, :, hh * ODC : (hh + 1) * ODC] = res.results[core]["out"]
    return out



# revision 38
# speedup vs baseline: 2.2998x; 2.2998x over previous
"""BertSelfAttention (relative_key + skim-mask softmax) Trainium2 kernel.

Sharding: 8 cores = 4 batches x 2 head-halves. Each core handles one batch
and 8 heads (columns h*64..h*64+63 of Q/K/V for its head-half).

Device pipeline per core (software-pipelined across heads; qd matmuls of
head h, scores of head h-2 and PV of head h-3 are interleaved on the PE
instruction stream):
  1. Q/K projections (bf16 matmuls), then V projection merged with the qd
     streams of heads 0-1.
  2. Per head: windowed qd[l, j] = q[l,:] @ dist_embT[:, j] matmuls; the
     PSUM->SBUF copies cast to fp8e4 (bias logits are small, so fp8 is
     safe); two DMAs per head whose DRAM-side access pattern absorbs the
     per-row diagonal shift, so DRAM element (l*RSTRIDE + l%128 + w)
     holds qd[l, AL(l//128) + w].
  3. Two contiguous DMAs per head read bias[l, r] = qd[l, 1024 - l + r]
     back: DRAM offset l*RSTRIDE + 128 + r (the skew cancels, r is
     contiguous, full DMA bandwidth).
  4. Per (head, R): eight plain fp8 matmuls with lhsT=bias tile and
     rhs=identity transpose the bias tiles directly into the f32 scores
     PSUM (start=True only on the first op touching each 2KB PSUM
     zero-region); two K=64 QK matmuls accumulate K^T Q on top
     (start=False); ACT computes exps = exp(scores + per-partition mask
     bias) in bf16.
  5. PV matmul with ones-column on V gives ctx^T and softmax denominators.
  6. Per-partition reciprocal + gpsimd broadcast normalize ctx^T; the
     output is written transposed ([head*64+d, l]) so the store DMA is
     contiguous; the host transposes back.
"""

import os
import sys

sys.path.insert(0, "/opt/trn_rl_repo")

import numpy as np
import ml_dtypes

import concourse.bass as bass
import concourse.tile as tile
from concourse import bacc, mybir
from concourse.bass_utils import run_bass_kernel_spmd

B, S, HID, H, D = 4, 1024, 1024, 16, 64
MAXP = 1024
EPS = 1e-8
NEG = -30.0          # additive bias for masked columns (exp -> ~1e-13)
HPC = 8              # heads per core
ODC = HPC * D        # 512 output dims per core
NJ = 2048            # reversed dist table columns
WIN = 1152           # qd j-window per 128-row l-chunk
RSTRIDE = 1280       # qdr DRAM row stride (>= WIN + 127 + 1)
SCALE = 1.0 / 8.0    # 1/sqrt(D)

BF16 = mybir.dt.bfloat16
F32 = mybir.dt.float32
NPBF16 = ml_dtypes.bfloat16

EXPF = mybir.ActivationFunctionType.Exp


def _body(nc, tc, s):
    """One full forward pass; s = dict of persistent tiles/handles."""
    dz_sb, mb_sb, id_sb = s["dz_sb"], s["mb_sb"], s["id_sb"]
    idf_sb = s["idf_sb"]
    QTb, KTb, Vb, qdr = s["QTb"], s["KTb"], s["Vb"], s["qdr"]

    # ---- stage 1 + stages 2-4, software-pipelined --------------------
    # Emission order: Q-proj, K-proj (serial); then V-proj merged with the
    # qd streams of heads 0-1; then per step hh: qd(hh) merged with
    # scores(hh-2) and PV(hh-3).  Each head's qd tiles round-trip DRAM
    # via one skewed write + one contiguous skewed read (bias[l, r]).
    def qk_proj(wq_sb, wk_sb, hT_sb, projp):
        for m in range(4):
            for sc in range(2):
                ps = projp.tile([128, 512], F32, tag="proj", name="psq")
                for k in range(8):
                    nc.tensor.matmul(
                        ps[:],
                        lhsT=wq_sb[:, k, m * 128 : (m + 1) * 128],
                        rhs=hT_sb[:, k, sc * 512 : (sc + 1) * 512],
                        start=(k == 0),
                        stop=(k == 7),
                    )
                nc.vector.tensor_scalar_mul(
                    QTb[:, m, sc * 512 : (sc + 1) * 512], ps[:], SCALE
                )
        for m in range(4):
            for sc in range(2):
                ps = projp.tile([128, 512], F32, tag="proj", name="psk")
                for k in range(8):
                    nc.tensor.matmul(
                        ps[:],
                        lhsT=wk_sb[:, k, m * 128 : (m + 1) * 128],
                        rhs=hT_sb[:, k, sc * 512 : (sc + 1) * 512],
                        start=(k == 0),
                        stop=(k == 7),
                    )
                nc.scalar.activation(
                    KTb[:, m, sc * 512 : (sc + 1) * 512], ps[:],
                    mybir.ActivationFunctionType.Copy,
                )

    def v_proj_stream(wv_sb, hT_sb, projvp):
        for sc in range(8):
            def op(sc=sc):
                ps = projvp.tile([128, 512], F32, tag="projv", name="psv")
                for k in range(8):
                    nc.tensor.matmul(
                        ps[:],
                        lhsT=hT_sb[:, k, sc * 128 : (sc + 1) * 128],
                        rhs=wv_sb[:, k, :],
                        start=(k == 0),
                        stop=(k == 7),
                    )
                eng = nc.scalar
                if eng is nc.scalar:
                    nc.scalar.activation(
                        Vb[:, sc, :, 0:D],
                        ps[:].rearrange("p (h dd) -> p h dd", dd=D),
                        mybir.ActivationFunctionType.Copy,
                    )
                else:
                    eng.tensor_copy(
                        out=Vb[:, sc, :, 0:D],
                        in_=ps[:].rearrange("p (h dd) -> p h dd", dd=D),
                    )
            yield op

    with (
        tc.tile_pool(name="qdp", bufs=3, space="PSUM") as qdp,
        tc.tile_pool(name="qdsbp", bufs=2) as qdsbp,
        tc.tile_pool(name="biasp", bufs=2) as biasp,
        tc.tile_pool(name="expsp", bufs=2) as expsp,
        tc.tile_pool(name="smallp", bufs=2) as smallp,
    ):
        bias_tiles = {}
        exps_tiles = {}

        def qd_stream(h):
            hp = h // 2
            hr = slice(0, 64) if h % 2 == 0 else slice(64, 128)
            qd_sb = qdsbp.tile([128, 8, WIN], BF16, tag="qd", name=f"qd{h}")
            bias_sb = biasp.tile([128, 8, S], BF16, tag="bias", name=f"bias{h}")
            bias_tiles[h] = bias_sb

            def rt_dma(Ls):
                # skewed write + contiguous skewed read for L in [Ls, Ls+4)
                dst = bass.AP(
                    tensor=qdr[h],
                    offset=Ls * 128 * RSTRIDE,
                    ap=[[RSTRIDE + 1, 128], [128 * RSTRIDE, 4], [1, WIN]],
                )
                nc.sync.dma_start(dst, qd_sb[:, Ls : Ls + 4, :])
                src = bass.AP(
                    tensor=qdr[h],
                    offset=Ls * 128 * RSTRIDE + 128,
                    ap=[[RSTRIDE, 128], [128 * RSTRIDE, 4], [1, S]],
                )
                nc.sync.dma_start(bias_sb[:, Ls : Ls + 4, :], src)

            for L in range(8):
                AL = 896 - 128 * L
                for ci, (n0, nn) in enumerate(
                    ((0, 512), (512, 512), (1024, 128))
                ):
                    def op(L=L, n0=n0, nn=nn, ci=ci, AL=AL):
                        ps = qdp.tile([128, 512], F32, tag="qdps", name="qdps")
                        nc.tensor.matmul(
                            ps[:, :nn],
                            lhsT=QTb[hr, hp, L * 128 : (L + 1) * 128],
                            rhs=dz_sb[hr, AL + n0 : AL + n0 + nn],
                            start=True,
                            stop=True,
                        )
                        dstv = qd_sb[:, L, n0 : n0 + nn]
                        use_act = ci == 2 or (ci == 1 and L % 4 == 3)
                        if use_act:
                            nc.scalar.activation(
                                dstv, ps[:, :nn],
                                mybir.ActivationFunctionType.Copy,
                            )
                        else:
                            nc.vector.tensor_copy(out=dstv, in_=ps[:, :nn])
                        if L == 3 and ci == 2:
                            rt_dma(0)
                        elif L == 7 and ci == 2:
                            rt_dma(4)
                    yield op

        def scores_stream(h, scoresp):
            hp = h // 2
            hr = slice(0, 64) if h % 2 == 0 else slice(64, 128)
            bias_sb = bias_tiles.pop(h)
            exps = expsp.tile([128, 8, S], BF16, tag="exps", name=f"exps{h}")
            exps_tiles[h] = exps
            for R in range(8):
                def op(R=R):
                    trb = trbp.tile([128, 1024], BF16, tag="trb", name="trb")
                    for i in range(8):
                        nc.tensor.transpose(
                            trb[:, i * 128 : (i + 1) * 128],
                            bias_sb[:, i, R * 128 : (R + 1) * 128],
                            id_sb[:],
                        )
                    sc = scoresp.tile(
                        [128, 1024], F32, tag="scores", name="sc_ps"
                    )
                    for lc in range(2):
                        nc.tensor.matmul(
                            sc[:, lc * 512 : (lc + 1) * 512],
                            lhsT=KTb[hr, hp, R * 128 : (R + 1) * 128],
                            rhs=QTb[hr, hp, lc * 512 : (lc + 1) * 512],
                            start=True,
                            stop=True,
                        )
                    nc.vector.tensor_add(sc[:], sc[:], trb[:])
                    nc.scalar.activation(
                        exps[:, R, :],
                        sc[:],
                        EXPF,
                        bias=mb_sb[:, R : R + 1],
                    )
                yield op

        def pv_stream(h, ctxp):
            exps = exps_tiles.pop(h)
            out_sb = smallp.tile([D, 2, 512], F32, tag="out", name=f"out{h}")
            lcs = (0, 1) if h < HPC - 1 else (1, 0)
            for lc in lcs:
                def op(lc=lc):
                    ct_ps = ctxp.tile([128, 512], F32, tag="ctx", name="ct_ps")
                    for R in range(8):
                        nc.tensor.matmul(
                            ct_ps[0 : D + 1, :],
                            lhsT=Vb[:, R, h, :],
                            rhs=exps[:, R, lc * 512 : (lc + 1) * 512],
                            start=(R == 0),
                            stop=(R == 7),
                        )
                    ctsb = smallp.tile(
                        [D + 1, 512], F32, tag="ctsb", name="ctsb"
                    )
                    if lc == 0:
                        nc.vector.tensor_copy(out=ctsb[:], in_=ct_ps[0 : D + 1, :])
                    else:
                        nc.scalar.activation(
                            ctsb[:], ct_ps[0 : D + 1, :],
                            mybir.ActivationFunctionType.Copy,
                        )
                    den = smallp.tile([1, 512], F32, tag="den", name="den")
                    nc.gpsimd.tensor_scalar_add(
                        den[:], ctsb[D : D + 1, :], EPS
                    )
                    rec = smallp.tile([1, 512], F32, tag="rec", name="rec")
                    nc.vector.reciprocal(rec[:], den[:])
                    rb = smallp.tile([D, 512], F32, tag="rb", name="rb")
                    nc.gpsimd.partition_broadcast(rb[:], rec[:], channels=D)
                    nc.gpsimd.tensor_mul(
                        out_sb[:, lc, :], ctsb[0:D, :], rb[:]
                    )
                    dst = bass.AP(
                        tensor=s["out"],
                        offset=h * D * S + lc * 512,
                        ap=[[S, D], [1, 512]],
                    )
                    nc.sync.dma_start(dst, out_sb[:, lc, :])
                yield op

        def merge(lists, weights=None):
            lists = [l for l in lists if l]
            if weights is None:
                weights = [1.0] * len(lists)
            idx = [0] * len(lists)
            total = sum(len(l) for l in lists)
            for _ in range(total):
                best, bf = None, None
                for j, l in enumerate(lists):
                    if idx[j] < len(l):
                        f = idx[j] / (len(l) * weights[j])
                        if bf is None or f < bf:
                            best, bf = j, f
                lists[best][idx[best]]()
                idx[best] += 1

        with tc.tile_pool(name="stg1b", bufs=1) as stg1b:
            hT_sb = stg1b.tile([128, HID // 128, S], BF16)
            wv_sb = stg1b.tile([128, HID // 128, ODC], BF16)
            with (
                tc.tile_pool(name="stg1a", bufs=1) as stg1a,
                tc.tile_pool(name="projp", bufs=4, space="PSUM") as projp,
            ):
                wq_sb = stg1a.tile([128, HID // 128, ODC], BF16)
                wq_r = s["wq"].ap().rearrange("(k p) o -> p k o", p=128)
                nc.sync.dma_start(wq_sb[:, 0:4, :], wq_r[:, 0:4, :])
                hT_r = s["hT"].ap().rearrange("(k p) s -> p k s", p=128)
                nc.sync.dma_start(hT_sb[:, 0:2, :], hT_r[:, 0:2, :])
                nc.sync.dma_start(wq_sb[:, 4:8, :], wq_r[:, 4:8, :])
                nc.sync.dma_start(hT_sb[:, 2:4, :], hT_r[:, 2:4, :])
                nc.sync.dma_start(hT_sb[:, 4:6, :], hT_r[:, 4:6, :])
                nc.sync.dma_start(hT_sb[:, 6:8, :], hT_r[:, 6:8, :])
                wk_sb = stg1a.tile([128, HID // 128, ODC], BF16)
                nc.sync.dma_start(
                    wk_sb[:],
                    s["wk"].ap().rearrange("(k p) o -> p k o", p=128),
                )
                nc.sync.dma_start(
                    wv_sb[:],
                    s["wv"].ap().rearrange("(k p) o -> p k o", p=128),
                )
                if s.get("load_consts"):
                    nc.sync.dma_start(s["dz_sb"][:], s["distZ"].ap())
                    nc.sync.dma_start(s["mb_sb"][:], s["mbias"].ap())
                    nc.sync.dma_start(s["id_sb"][:], s["ident"].ap())
                    nc.sync.dma_start(s["idf_sb"][:], s["identf"].ap())
                    s["load_consts"] = False
                qk_proj(wq_sb, wk_sb, hT_sb, projp)
            with tc.tile_pool(name="projvp", bufs=2, space="PSUM") as projvp:
                merge(
                    [
                        list(v_proj_stream(wv_sb, hT_sb, projvp)),
                        list(qd_stream(0)),
                        list(qd_stream(1)),
                    ],
                    weights=[1.0, 4.0, 1.6],
                )

        with (
            tc.tile_pool(name="scoresp", bufs=2, space="PSUM") as scoresp,
            tc.tile_pool(name="ctxp", bufs=1, space="PSUM") as ctxp,
        ):
            for hh in range(2, HPC + 3):
                streams = []
                weights = []
                if hh < HPC:
                    streams.append(list(qd_stream(hh)))
                    weights.append(2.5 if hh == 2 else 1.2)
                if hh < HPC + 2:
                    streams.append(list(scores_stream(hh - 2, scoresp)))
                    weights.append(0.8 if hh == 2 else (1.5 if hh >= HPC else 1.0))
                if hh >= 3:
                    streams.append(list(pv_stream(hh - 3, ctxp)))
                    weights.append(1.0)
                merge(streams, weights)


def build_program(n_reps=1):
    nc = bacc.Bacc(trn_type="TRN2", target_bir_lowering=False, debug=False)

    hT = nc.dram_tensor("hT", [HID, S], BF16, kind="ExternalInput")
    wq = nc.dram_tensor("wq", [HID, ODC], BF16, kind="ExternalInput")
    wk = nc.dram_tensor("wk", [HID, ODC], BF16, kind="ExternalInput")
    wv = nc.dram_tensor("wv", [HID, ODC], BF16, kind="ExternalInput")
    distZ = nc.dram_tensor("distZ", [128, NJ], BF16, kind="ExternalInput")
    mbias = nc.dram_tensor("mbias", [128, 8], F32, kind="ExternalInput")
    ident = nc.dram_tensor("ident", [128, 128], BF16, kind="ExternalInput")
    identf = nc.dram_tensor("identf", [128, 128], F32, kind="ExternalInput")
    # output stays transposed per head: row h*64+d holds ctx[:, h, d]
    out = nc.dram_tensor("out", [HPC * D, S], F32, kind="ExternalOutput")

    # per-head DRAM scratch for the skew-compacted qd rows
    qdr = [nc.dram_tensor(f"qdr{h}", [S * RSTRIDE], BF16) for h in range(HPC)]

    with tile.TileContext(nc) as tc:
        with tc.tile_pool(name="singles", bufs=1) as singles:
            dz_sb = singles.tile([128, NJ], BF16)
            mb_sb = singles.tile([128, 8], F32)
            id_sb = singles.tile([128, 128], BF16)
            idf_sb = singles.tile([128, 128], F32)

            QTb = singles.tile([128, 4, S], BF16)   # [od%128, od//128, s]
            KTb = singles.tile([128, 4, S], BF16)
            # V natural with ones column: [s%128, s//128, h, 65]
            Vb = singles.tile([128, 8, HPC, D + 1], BF16)

            nc.vector.memset(Vb[:, :, :, D : D + 1], 1.0)

            state = dict(
                hT=hT, wq=wq, wk=wk, wv=wv,
                distZ=distZ, mbias=mbias, ident=ident, identf=identf,
                out=out,
                dz_sb=dz_sb, mb_sb=mb_sb, id_sb=id_sb, idf_sb=idf_sb,
                QTb=QTb, KTb=KTb, Vb=Vb, qdr=qdr,
                load_consts=True,
            )
            for _rep in range(n_reps):
                state["load_consts"] = True
                _body(nc, tc, state)

    nc.compile()
    return nc


def make_core_inputs(hidden_states, attention_mask, skim_mask, Wq, Wk, Wv, dist_emb):
    """Host-side prep: returns list of 8 in_maps."""
    hidden_states = np.asarray(hidden_states, np.float32)
    attention_mask = np.asarray(attention_mask, np.float32)
    skim_mask = np.asarray(skim_mask)
    Wq = np.asarray(Wq, np.float32)
    Wk = np.asarray(Wk, np.float32)
    Wv = np.asarray(Wv, np.float32)
    dist_emb = np.asarray(dist_emb, np.float32)

    # reversed dist table: dz[d, j] = dist_emb[2047 - j, d] for j >= 1,
    # col 0 = 0; replicated in both partition halves so even heads use rows
    # 0-63 and odd heads rows 64-127.
    dz = np.zeros((128, NJ), np.float32)
    tmp = dist_emb[::-1].T  # [64, 2047]; tmp[d, i] = dist_emb[2046 - i, d]
    dz[0:64, 1:NJ] = tmp
    dz[64:128, 1:NJ] = tmp
    dz = np.ascontiguousarray(dz.astype(NPBF16))

    ident = np.ascontiguousarray(np.eye(128, dtype=NPBF16))
    identf = np.ascontiguousarray(np.eye(128, dtype=np.float32))

    in_maps = []
    for core in range(8):
        b, hh = core // 2, core % 2
        cols = slice(hh * ODC, (hh + 1) * ODC)
        hT = np.ascontiguousarray(hidden_states[b].T.astype(NPBF16))
        mb = (
            attention_mask[b, 0, 0, :] + NEG * (1.0 - skim_mask[b].astype(np.float32))
        ).astype(np.float32)
        in_maps.append(
            {
                "hT": hT,
                "wq": np.ascontiguousarray(Wq[:, cols].astype(NPBF16)),
                "wk": np.ascontiguousarray(Wk[:, cols].astype(NPBF16)),
                "wv": np.ascontiguousarray(Wv[:, cols].astype(NPBF16)),
                "distZ": dz,
                "mbias": np.ascontiguousarray(mb.reshape(8, 128).T),
                "ident": ident,
                "identf": identf,
            }
        )
    return in_maps


def kernel(
    hidden_states,
    attention_mask,
    skim_mask,
    Wq,
    bq,
    Wk,
    bk,
    Wv,
    bv,
    dist_emb,
):
    in_maps = make_core_inputs(
        hidden_states, attention_mask, skim_mask, Wq, Wk, Wv, dist_emb
    )
    nc = build_program()
    res = run_bass_kernel_spmd(nc, in_maps, core_ids=list(range(8)))
    out = np.zeros((B, S, HID), np.float32)
    for core in range(8):
        b, hh = core // 2, core % 2
        out[b, :, hh * ODC : (hh + 1) * ODC] = res.results[core]["out"].T
    return out


# revision 43
# speedup vs baseline: 2.3039x; 1.0018x over previous
"""BertSelfAttention (relative_key + skim-mask softmax) Trainium2 kernel.

Sharding: 8 cores = 4 batches x 2 head-halves. Each core handles one batch
and 8 heads (columns h*64..h*64+63 of Q/K/V for its head-half).

Device pipeline per core (software-pipelined across heads; qd matmuls of
head h, scores of head h-2 and PV of head h-3 are interleaved on the PE
instruction stream):
  1. Q/K projections (bf16 matmuls), then V projection merged with the qd
     streams of heads 0-1.
  2. Per head: windowed qd[l, j] = q[l,:] @ dist_embT[:, j] matmuls; the
     PSUM->SBUF copies cast to fp8e4 (bias logits are small, so fp8 is
     safe); two DMAs per head whose DRAM-side access pattern absorbs the
     per-row diagonal shift, so DRAM element (l*RSTRIDE + l%128 + w)
     holds qd[l, AL(l//128) + w].
  3. Two contiguous DMAs per head read bias[l, r] = qd[l, 1024 - l + r]
     back: DRAM offset l*RSTRIDE + 128 + r (the skew cancels, r is
     contiguous, full DMA bandwidth).
  4. Per (head, R): eight plain fp8 matmuls with lhsT=bias tile and
     rhs=identity transpose the bias tiles directly into the f32 scores
     PSUM (start=True only on the first op touching each 2KB PSUM
     zero-region); two K=64 QK matmuls accumulate K^T Q on top
     (start=False); ACT computes exps = exp(scores + per-partition mask
     bias) in bf16.
  5. PV matmul with ones-column on V gives ctx^T and softmax denominators.
  6. Per-partition reciprocal + gpsimd broadcast normalize ctx^T; the
     output is written transposed ([head*64+d, l]) so the store DMA is
     contiguous; the host transposes back.
"""

import os
import sys

sys.path.insert(0, "/opt/trn_rl_repo")

import numpy as np
import ml_dtypes

import concourse.bass as bass
import concourse.tile as tile
from concourse import bacc, mybir
from concourse.bass_utils import run_bass_kernel_spmd

B, S, HID, H, D = 4, 1024, 1024, 16, 64
MAXP = 1024
EPS = 1e-8
NEG = -30.0          # additive bias for masked columns (exp -> ~1e-13)
HPC = 8              # heads per core
ODC = HPC * D        # 512 output dims per core
NJ = 2048            # reversed dist table columns
WIN = 1152           # qd j-window per 128-row l-chunk
RSTRIDE = 1280       # qdr DRAM row stride (>= WIN + 127 + 1)
SCALE = 1.0 / 8.0    # 1/sqrt(D)

BF16 = mybir.dt.bfloat16
F32 = mybir.dt.float32
NPBF16 = ml_dtypes.bfloat16

EXPF = mybir.ActivationFunctionType.Exp


def _body(nc, tc, s):
    """One full forward pass; s = dict of persistent tiles/handles."""
    dz_sb, mb_sb, id_sb = s["dz_sb"], s["mb_sb"], s["id_sb"]
    idf_sb = s["idf_sb"]
    QTb, KTb, Vb, qdr = s["QTb"], s["KTb"], s["Vb"], s["qdr"]

    # ---- stage 1 + stages 2-4, software-pipelined --------------------
    # Emission order: Q-proj, K-proj (serial); then V-proj merged with the
    # qd streams of heads 0-1; then per step hh: qd(hh) merged with
    # scores(hh-2) and PV(hh-3).  Each head's qd tiles round-trip DRAM
    # via one skewed write + one contiguous skewed read (bias[l, r]).
    def qk_proj(wq_sb, wk_sb, hT_sb, projp):
        for m in range(4):
            for sc in range(2):
                ps = projp.tile([128, 512], F32, tag="proj", name="psq")
                for k in range(8):
                    nc.tensor.matmul(
                        ps[:],
                        lhsT=wq_sb[:, k, m * 128 : (m + 1) * 128],
                        rhs=hT_sb[:, k, sc * 512 : (sc + 1) * 512],
                        start=(k == 0),
                        stop=(k == 7),
                    )
                nc.vector.tensor_scalar_mul(
                    QTb[:, m, sc * 512 : (sc + 1) * 512], ps[:], SCALE
                )
        for m in range(4):
            for sc in range(2):
                ps = projp.tile([128, 512], F32, tag="proj", name="psk")
                for k in range(8):
                    nc.tensor.matmul(
                        ps[:],
                        lhsT=wk_sb[:, k, m * 128 : (m + 1) * 128],
                        rhs=hT_sb[:, k, sc * 512 : (sc + 1) * 512],
                        start=(k == 0),
                        stop=(k == 7),
                    )
                nc.scalar.activation(
                    KTb[:, m, sc * 512 : (sc + 1) * 512], ps[:],
                    mybir.ActivationFunctionType.Copy,
                )

    def v_proj_stream(wv_sb, hT_sb, projvp):
        for sc in range(8):
            def op(sc=sc):
                ps = projvp.tile([128, 512], F32, tag="projv", name="psv")
                for k in range(8):
                    nc.tensor.matmul(
                        ps[:],
                        lhsT=hT_sb[:, k, sc * 128 : (sc + 1) * 128],
                        rhs=wv_sb[:, k, :],
                        start=(k == 0),
                        stop=(k == 7),
                    )
                eng = nc.scalar
                if eng is nc.scalar:
                    nc.scalar.activation(
                        Vb[:, sc, :, 0:D],
                        ps[:].rearrange("p (h dd) -> p h dd", dd=D),
                        mybir.ActivationFunctionType.Copy,
                    )
                else:
                    eng.tensor_copy(
                        out=Vb[:, sc, :, 0:D],
                        in_=ps[:].rearrange("p (h dd) -> p h dd", dd=D),
                    )
            yield op

    with (
        tc.tile_pool(name="qdp", bufs=3, space="PSUM") as qdp,
        tc.tile_pool(name="qdsbp", bufs=2) as qdsbp,
        tc.tile_pool(name="biasp", bufs=2) as biasp,
        tc.tile_pool(name="expsp", bufs=2) as expsp,
        tc.tile_pool(name="smallp", bufs=2) as smallp,
    ):
        bias_tiles = {}
        exps_tiles = {}

        def qd_stream(h):
            hp = h // 2
            hr = slice(0, 64) if h % 2 == 0 else slice(64, 128)
            qd_sb = qdsbp.tile([128, 8, WIN], BF16, tag="qd", name=f"qd{h}")
            bias_sb = biasp.tile([128, 8, S], BF16, tag="bias", name=f"bias{h}")
            bias_tiles[h] = bias_sb

            def rt_dma(Ls):
                # skewed write + contiguous skewed read for L in [Ls, Ls+4)
                dst = bass.AP(
                    tensor=qdr[h],
                    offset=Ls * 128 * RSTRIDE,
                    ap=[[RSTRIDE + 1, 128], [128 * RSTRIDE, 4], [1, WIN]],
                )
                nc.sync.dma_start(dst, qd_sb[:, Ls : Ls + 4, :])
                src = bass.AP(
                    tensor=qdr[h],
                    offset=Ls * 128 * RSTRIDE + 128,
                    ap=[[RSTRIDE, 128], [128 * RSTRIDE, 4], [1, S]],
                )
                nc.sync.dma_start(bias_sb[:, Ls : Ls + 4, :], src)

            for L in range(8):
                AL = 896 - 128 * L
                for ci, (n0, nn) in enumerate(
                    ((0, 512), (512, 512), (1024, 128))
                ):
                    def op(L=L, n0=n0, nn=nn, ci=ci, AL=AL):
                        ps = qdp.tile([128, 512], F32, tag="qdps", name="qdps")
                        nc.tensor.matmul(
                            ps[:, :nn],
                            lhsT=QTb[hr, hp, L * 128 : (L + 1) * 128],
                            rhs=dz_sb[hr, AL + n0 : AL + n0 + nn],
                            start=True,
                            stop=True,
                        )
                        dstv = qd_sb[:, L, n0 : n0 + nn]
                        use_act = ci == 2 or (ci == 1 and L % 4 == 3)
                        if use_act:
                            nc.scalar.activation(
                                dstv, ps[:, :nn],
                                mybir.ActivationFunctionType.Copy,
                            )
                        else:
                            nc.vector.tensor_copy(out=dstv, in_=ps[:, :nn])
                        if L == 3 and ci == 2:
                            rt_dma(0)
                        elif L == 7 and ci == 2:
                            rt_dma(4)
                    yield op

        def scores_stream(h, scoresp):
            hp = h // 2
            hr = slice(0, 64) if h % 2 == 0 else slice(64, 128)
            bias_sb = bias_tiles.pop(h)
            exps = expsp.tile([128, 8, S], BF16, tag="exps", name=f"exps{h}")
            exps_tiles[h] = exps
            for R in range(8):
                def op(R=R):
                    trb = trbp.tile([128, 1024], BF16, tag="trb", name="trb")
                    for i in range(8):
                        nc.tensor.transpose(
                            trb[:, i * 128 : (i + 1) * 128],
                            bias_sb[:, i, R * 128 : (R + 1) * 128],
                            id_sb[:],
                        )
                    sc = scoresp.tile(
                        [128, 1024], F32, tag="scores", name="sc_ps"
                    )
                    for lc in range(2):
                        nc.tensor.matmul(
                            sc[:, lc * 512 : (lc + 1) * 512],
                            lhsT=KTb[hr, hp, R * 128 : (R + 1) * 128],
                            rhs=QTb[hr, hp, lc * 512 : (lc + 1) * 512],
                            start=True,
                            stop=True,
                        )
                    nc.vector.tensor_add(sc[:], sc[:], trb[:])
                    nc.scalar.activation(
                        exps[:, R, :],
                        sc[:],
                        EXPF,
                        bias=mb_sb[:, R : R + 1],
                    )
                yield op

        def pv_stream(h, ctxp):
            exps = exps_tiles.pop(h)
            out_sb = smallp.tile([D, 2, 512], F32, tag="out", name=f"out{h}")
            lcs = (0, 1) if h < HPC - 1 else (1, 0)
            for li, lc in enumerate(lcs):
                # the very last tile has no PSUM successor: skip the ctsb
                # staging hop and run a shorter DVE-direct chain
                last = h == HPC - 1 and li == 1
                def op(lc=lc, last=last):
                    ct_ps = ctxp.tile([128, 512], F32, tag="ctx", name="ct_ps")
                    for R in range(8):
                        nc.tensor.matmul(
                            ct_ps[0 : D + 1, :],
                            lhsT=Vb[:, R, h, :],
                            rhs=exps[:, R, lc * 512 : (lc + 1) * 512],
                            start=(R == 0),
                            stop=(R == 7),
                        )
                    den = smallp.tile([1, 512], F32, tag="den", name="den")
                    rec = smallp.tile([1, 512], F32, tag="rec", name="rec")
                    rb = smallp.tile([D, 512], F32, tag="rb", name="rb")
                    if last:
                        nc.vector.tensor_scalar_add(
                            den[:], ct_ps[D : D + 1, :], EPS
                        )
                        nc.vector.reciprocal(rec[:], den[:])
                        nc.gpsimd.partition_broadcast(rb[:], rec[:], channels=D)
                        nc.vector.tensor_mul(
                            out_sb[:, lc, :], ct_ps[0:D, :], rb[:]
                        )
                    else:
                        ctsb = smallp.tile(
                            [D + 1, 512], F32, tag="ctsb", name="ctsb"
                        )
                        if lc == 0:
                            nc.vector.tensor_copy(
                                out=ctsb[:], in_=ct_ps[0 : D + 1, :]
                            )
                        else:
                            nc.scalar.activation(
                                ctsb[:], ct_ps[0 : D + 1, :],
                                mybir.ActivationFunctionType.Copy,
                            )
                        nc.gpsimd.tensor_scalar_add(
                            den[:], ctsb[D : D + 1, :], EPS
                        )
                        nc.vector.reciprocal(rec[:], den[:])
                        nc.gpsimd.partition_broadcast(rb[:], rec[:], channels=D)
                        nc.gpsimd.tensor_mul(
                            out_sb[:, lc, :], ctsb[0:D, :], rb[:]
                        )
                    dst = bass.AP(
                        tensor=s["out"],
                        offset=h * D * S + lc * 512,
                        ap=[[S, D], [1, 512]],
                    )
                    nc.sync.dma_start(dst, out_sb[:, lc, :])
                yield op

        def merge(lists, weights=None):
            lists = [l for l in lists if l]
            if weights is None:
                weights = [1.0] * len(lists)
            idx = [0] * len(lists)
            total = sum(len(l) for l in lists)
            for _ in range(total):
                best, bf = None, None
                for j, l in enumerate(lists):
                    if idx[j] < len(l):
                        f = idx[j] / (len(l) * weights[j])
                        if bf is None or f < bf:
                            best, bf = j, f
                lists[best][idx[best]]()
                idx[best] += 1

        with tc.tile_pool(name="stg1b", bufs=1) as stg1b:
            hT_sb = stg1b.tile([128, HID // 128, S], BF16)
            wv_sb = stg1b.tile([128, HID // 128, ODC], BF16)
            with (
                tc.tile_pool(name="stg1a", bufs=1) as stg1a,
                tc.tile_pool(name="projp", bufs=4, space="PSUM") as projp,
            ):
                wq_sb = stg1a.tile([128, HID // 128, ODC], BF16)
                wq_r = s["wq"].ap().rearrange("(k p) o -> p k o", p=128)
                nc.sync.dma_start(wq_sb[:, 0:4, :], wq_r[:, 0:4, :])
                hT_r = s["hT"].ap().rearrange("(k p) s -> p k s", p=128)
                nc.sync.dma_start(hT_sb[:, 0:2, :], hT_r[:, 0:2, :])
                nc.sync.dma_start(wq_sb[:, 4:8, :], wq_r[:, 4:8, :])
                nc.sync.dma_start(hT_sb[:, 2:4, :], hT_r[:, 2:4, :])
                nc.sync.dma_start(hT_sb[:, 4:6, :], hT_r[:, 4:6, :])
                nc.sync.dma_start(hT_sb[:, 6:8, :], hT_r[:, 6:8, :])
                wk_sb = stg1a.tile([128, HID // 128, ODC], BF16)
                nc.sync.dma_start(
                    wk_sb[:],
                    s["wk"].ap().rearrange("(k p) o -> p k o", p=128),
                )
                nc.sync.dma_start(
                    wv_sb[:],
                    s["wv"].ap().rearrange("(k p) o -> p k o", p=128),
                )
                if s.get("load_consts"):
                    nc.sync.dma_start(s["dz_sb"][:], s["distZ"].ap())
                    nc.sync.dma_start(s["mb_sb"][:], s["mbias"].ap())
                    nc.sync.dma_start(s["id_sb"][:], s["ident"].ap())
                    nc.sync.dma_start(s["idf_sb"][:], s["identf"].ap())
                    s["load_consts"] = False
                qk_proj(wq_sb, wk_sb, hT_sb, projp)
            with tc.tile_pool(name="projvp", bufs=2, space="PSUM") as projvp:
                merge(
                    [
                        list(v_proj_stream(wv_sb, hT_sb, projvp)),
                        list(qd_stream(0)),
                        list(qd_stream(1)),
                    ],
                    weights=[1.0, 4.0, 1.6],
                )

        with (
            tc.tile_pool(name="scoresp", bufs=2, space="PSUM") as scoresp,
            tc.tile_pool(name="ctxp", bufs=1, space="PSUM") as ctxp,
        ):
            for hh in range(2, HPC + 3):
                streams = []
                weights = []
                if hh < HPC:
                    streams.append(list(qd_stream(hh)))
                    weights.append(2.5 if hh == 2 else 1.2)
                if hh < HPC + 2:
                    streams.append(list(scores_stream(hh - 2, scoresp)))
                    weights.append(0.8 if hh == 2 else (1.5 if hh >= HPC else 1.0))
                if hh >= 3:
                    streams.append(list(pv_stream(hh - 3, ctxp)))
                    weights.append(1.0)
                merge(streams, weights)


def build_program(n_reps=1):
    nc = bacc.Bacc(trn_type="TRN2", target_bir_lowering=False, debug=False)

    hT = nc.dram_tensor("hT", [HID, S], BF16, kind="ExternalInput")
    wq = nc.dram_tensor("wq", [HID, ODC], BF16, kind="ExternalInput")
    wk = nc.dram_tensor("wk", [HID, ODC], BF16, kind="ExternalInput")
    wv = nc.dram_tensor("wv", [HID, ODC], BF16, kind="ExternalInput")
    distZ = nc.dram_tensor("distZ", [128, NJ], BF16, kind="ExternalInput")
    mbias = nc.dram_tensor("mbias", [128, 8], F32, kind="ExternalInput")
    ident = nc.dram_tensor("ident", [128, 128], BF16, kind="ExternalInput")
    identf = nc.dram_tensor("identf", [128, 128], F32, kind="ExternalInput")
    # output stays transposed per head: row h*64+d holds ctx[:, h, d]
    out = nc.dram_tensor("out", [HPC * D, S], F32, kind="ExternalOutput")

    # per-head DRAM scratch for the skew-compacted qd rows
    qdr = [nc.dram_tensor(f"qdr{h}", [S * RSTRIDE], BF16) for h in range(HPC)]

    with tile.TileContext(nc) as tc:
        with tc.tile_pool(name="singles", bufs=1) as singles:
            dz_sb = singles.tile([128, NJ], BF16)
            mb_sb = singles.tile([128, 8], F32)
            id_sb = singles.tile([128, 128], BF16)
            idf_sb = singles.tile([128, 128], F32)

            QTb = singles.tile([128, 4, S], BF16)   # [od%128, od//128, s]
            KTb = singles.tile([128, 4, S], BF16)
            # V natural with ones column: [s%128, s//128, h, 65]
            Vb = singles.tile([128, 8, HPC, D + 1], BF16)

            nc.vector.memset(Vb[:, :, :, D : D + 1], 1.0)

            state = dict(
                hT=hT, wq=wq, wk=wk, wv=wv,
                distZ=distZ, mbias=mbias, ident=ident, identf=identf,
                out=out,
                dz_sb=dz_sb, mb_sb=mb_sb, id_sb=id_sb, idf_sb=idf_sb,
                QTb=QTb, KTb=KTb, Vb=Vb, qdr=qdr,
                load_consts=True,
            )
            for _rep in range(n_reps):
                state["load_consts"] = True
                _body(nc, tc, state)

    nc.compile()
    return nc


def make_core_inputs(hidden_states, attention_mask, skim_mask, Wq, Wk, Wv, dist_emb):
    """Host-side prep: returns list of 8 in_maps."""
    hidden_states = np.asarray(hidden_states, np.float32)
    attention_mask = np.asarray(attention_mask, np.float32)
    skim_mask = np.asarray(skim_mask)
    Wq = np.asarray(Wq, np.float32)
    Wk = np.asarray(Wk, np.float32)
    Wv = np.asarray(Wv, np.float32)
    dist_emb = np.asarray(dist_emb, np.float32)

    # reversed dist table: dz[d, j] = dist_emb[2047 - j, d] for j >= 1,
    # col 0 = 0; replicated in both partition halves so even heads use rows
    # 0-63 and odd heads rows 64-127.
    dz = np.zeros((128, NJ), np.float32)
    tmp = dist_emb[::-1].T  # [64, 2047]; tmp[d, i] = dist_emb[2046 - i, d]
    dz[0:64, 1:NJ] = tmp
    dz[64:128, 1:NJ] = tmp
    dz = np.ascontiguousarray(dz.astype(NPBF16))

    ident = np.ascontiguousarray(np.eye(128, dtype=NPBF16))
    identf = np.ascontiguousarray(np.eye(128, dtype=np.float32))

    in_maps = []
    for core in range(8):
        b, hh = core // 2, core % 2
        cols = slice(hh * ODC, (hh + 1) * ODC)
        hT = np.ascontiguousarray(hidden_states[b].T.astype(NPBF16))
        mb = (
            attention_mask[b, 0, 0, :] + NEG * (1.0 - skim_mask[b].astype(np.float32))
        ).astype(np.float32)
        in_maps.append(
            {
                "hT": hT,
                "wq": np.ascontiguousarray(Wq[:, cols].astype(NPBF16)),
                "wk": np.ascontiguousarray(Wk[:, cols].astype(NPBF16)),
                "wv": np.ascontiguousarray(Wv[:, cols].astype(NPBF16)),
                "distZ": dz,
                "mbias": np.ascontiguousarray(mb.reshape(8, 128).T),
                "ident": ident,
                "identf": identf,
            }
        )
    return in_maps


def kernel(
    hidden_states,
    attention_mask,
    skim_mask,
    Wq,
    bq,
    Wk,
    bk,
    Wv,
    bv,
    dist_emb,
):
    in_maps = make_core_inputs(
        hidden_states, attention_mask, skim_mask, Wq, Wk, Wv, dist_emb
    )
    nc = build_program()
    res = run_bass_kernel_spmd(nc, in_maps, core_ids=list(range(8)))
    out = np.zeros((B, S, HID), np.float32)
    for core in range(8):
        b, hh = core // 2, core % 2
        out[b, :, hh * ODC : (hh + 1) * ODC] = res.results[core]["out"].T
    return out


# revision 44
# speedup vs baseline: 2.3047x; 1.0003x over previous
"""BertSelfAttention (relative_key + skim-mask softmax) Trainium2 kernel.

Sharding: 8 cores = 4 batches x 2 head-halves. Each core handles one batch
and 8 heads (columns h*64..h*64+63 of Q/K/V for its head-half).

Device pipeline per core (software-pipelined across heads; qd matmuls of
head h, scores of head h-2 and PV of head h-3 are interleaved on the PE
instruction stream):
  1. Q/K projections (bf16 matmuls), then V projection merged with the qd
     streams of heads 0-1.
  2. Per head: windowed qd[l, j] = q[l,:] @ dist_embT[:, j] matmuls; the
     PSUM->SBUF copies cast to fp8e4 (bias logits are small, so fp8 is
     safe); two DMAs per head whose DRAM-side access pattern absorbs the
     per-row diagonal shift, so DRAM element (l*RSTRIDE + l%128 + w)
     holds qd[l, AL(l//128) + w].
  3. Two contiguous DMAs per head read bias[l, r] = qd[l, 1024 - l + r]
     back: DRAM offset l*RSTRIDE + 128 + r (the skew cancels, r is
     contiguous, full DMA bandwidth).
  4. Per (head, R): eight plain fp8 matmuls with lhsT=bias tile and
     rhs=identity transpose the bias tiles directly into the f32 scores
     PSUM (start=True only on the first op touching each 2KB PSUM
     zero-region); two K=64 QK matmuls accumulate K^T Q on top
     (start=False); ACT computes exps = exp(scores + per-partition mask
     bias) in bf16.
  5. PV matmul with ones-column on V gives ctx^T and softmax denominators.
  6. Per-partition reciprocal + gpsimd broadcast normalize ctx^T; the
     output is written transposed ([head*64+d, l]) so the store DMA is
     contiguous; the host transposes back.
"""

import os
import sys

sys.path.insert(0, "/opt/trn_rl_repo")

import numpy as np
import ml_dtypes

import concourse.bass as bass
import concourse.tile as tile
from concourse import bacc, mybir
from concourse.bass_utils import run_bass_kernel_spmd

B, S, HID, H, D = 4, 1024, 1024, 16, 64
MAXP = 1024
EPS = 1e-8
NEG = -30.0          # additive bias for masked columns (exp -> ~1e-13)
HPC = 8              # heads per core
ODC = HPC * D        # 512 output dims per core
NJ = 2048            # reversed dist table columns
WIN = 1152           # qd j-window per 128-row l-chunk
RSTRIDE = 1280       # qdr DRAM row stride (>= WIN + 127 + 1)
SCALE = 1.0 / 8.0    # 1/sqrt(D)

BF16 = mybir.dt.bfloat16
F32 = mybir.dt.float32
NPBF16 = ml_dtypes.bfloat16

EXPF = mybir.ActivationFunctionType.Exp


def _body(nc, tc, s):
    """One full forward pass; s = dict of persistent tiles/handles."""
    dz_sb, mb_sb, id_sb = s["dz_sb"], s["mb_sb"], s["id_sb"]
    idf_sb = s["idf_sb"]
    QTb, KTb, Vb, qdr = s["QTb"], s["KTb"], s["Vb"], s["qdr"]

    # ---- stage 1 + stages 2-4, software-pipelined --------------------
    # Emission order: Q-proj, K-proj (serial); then V-proj merged with the
    # qd streams of heads 0-1; then per step hh: qd(hh) merged with
    # scores(hh-2) and PV(hh-3).  Each head's qd tiles round-trip DRAM
    # via one skewed write + one contiguous skewed read (bias[l, r]).
    def qk_proj(wq_sb, wk_sb, hT_sb, projp):
        for m in range(4):
            for sc in range(2):
                ps = projp.tile([128, 512], F32, tag="proj", name="psq")
                for k in range(8):
                    nc.tensor.matmul(
                        ps[:],
                        lhsT=wq_sb[:, k, m * 128 : (m + 1) * 128],
                        rhs=hT_sb[:, k, sc * 512 : (sc + 1) * 512],
                        start=(k == 0),
                        stop=(k == 7),
                    )
                nc.vector.tensor_scalar_mul(
                    QTb[:, m, sc * 512 : (sc + 1) * 512], ps[:], SCALE
                )
        for m in range(4):
            for sc in range(2):
                ps = projp.tile([128, 512], F32, tag="proj", name="psk")
                for k in range(8):
                    nc.tensor.matmul(
                        ps[:],
                        lhsT=wk_sb[:, k, m * 128 : (m + 1) * 128],
                        rhs=hT_sb[:, k, sc * 512 : (sc + 1) * 512],
                        start=(k == 0),
                        stop=(k == 7),
                    )
                nc.scalar.activation(
                    KTb[:, m, sc * 512 : (sc + 1) * 512], ps[:],
                    mybir.ActivationFunctionType.Copy,
                )

    def v_proj_stream(wv_sb, hT_sb, projvp):
        for sc in range(8):
            def op(sc=sc):
                ps = projvp.tile([128, 512], F32, tag="projv", name="psv")
                for k in range(8):
                    nc.tensor.matmul(
                        ps[:],
                        lhsT=hT_sb[:, k, sc * 128 : (sc + 1) * 128],
                        rhs=wv_sb[:, k, :],
                        start=(k == 0),
                        stop=(k == 7),
                    )
                eng = nc.scalar
                if eng is nc.scalar:
                    nc.scalar.activation(
                        Vb[:, sc, :, 0:D],
                        ps[:].rearrange("p (h dd) -> p h dd", dd=D),
                        mybir.ActivationFunctionType.Copy,
                    )
                else:
                    eng.tensor_copy(
                        out=Vb[:, sc, :, 0:D],
                        in_=ps[:].rearrange("p (h dd) -> p h dd", dd=D),
                    )
            yield op

    with (
        tc.tile_pool(name="qdp", bufs=3, space="PSUM") as qdp,
        tc.tile_pool(name="qdsbp", bufs=2) as qdsbp,
        tc.tile_pool(name="biasp", bufs=2) as biasp,
        tc.tile_pool(name="expsp", bufs=2) as expsp,
        tc.tile_pool(name="smallp", bufs=2) as smallp,
    ):
        bias_tiles = {}
        exps_tiles = {}

        def qd_stream(h):
            hp = h // 2
            hr = slice(0, 64) if h % 2 == 0 else slice(64, 128)
            qd_sb = qdsbp.tile([128, 8, WIN], BF16, tag="qd", name=f"qd{h}")
            bias_sb = biasp.tile([128, 8, S], BF16, tag="bias", name=f"bias{h}")
            bias_tiles[h] = bias_sb

            def rt_dma(Ls):
                # skewed write + contiguous skewed read for L in [Ls, Ls+4)
                dst = bass.AP(
                    tensor=qdr[h],
                    offset=Ls * 128 * RSTRIDE,
                    ap=[[RSTRIDE + 1, 128], [128 * RSTRIDE, 4], [1, WIN]],
                )
                nc.sync.dma_start(dst, qd_sb[:, Ls : Ls + 4, :])
                src = bass.AP(
                    tensor=qdr[h],
                    offset=Ls * 128 * RSTRIDE + 128,
                    ap=[[RSTRIDE, 128], [128 * RSTRIDE, 4], [1, S]],
                )
                nc.sync.dma_start(bias_sb[:, Ls : Ls + 4, :], src)

            for L in range(8):
                AL = 896 - 128 * L
                for ci, (n0, nn) in enumerate(
                    ((0, 512), (512, 512), (1024, 128))
                ):
                    def op(L=L, n0=n0, nn=nn, ci=ci, AL=AL):
                        ps = qdp.tile([128, 512], F32, tag="qdps", name="qdps")
                        nc.tensor.matmul(
                            ps[:, :nn],
                            lhsT=QTb[hr, hp, L * 128 : (L + 1) * 128],
                            rhs=dz_sb[hr, AL + n0 : AL + n0 + nn],
                            start=True,
                            stop=True,
                        )
                        dstv = qd_sb[:, L, n0 : n0 + nn]
                        use_act = ci == 2 or (ci == 1 and L % 4 == 3)
                        if use_act:
                            nc.scalar.activation(
                                dstv, ps[:, :nn],
                                mybir.ActivationFunctionType.Copy,
                            )
                        else:
                            nc.vector.tensor_copy(out=dstv, in_=ps[:, :nn])
                        if L == 3 and ci == 2:
                            rt_dma(0)
                        elif L == 7 and ci == 2:
                            rt_dma(4)
                    yield op

        def scores_stream(h, scoresp):
            hp = h // 2
            hr = slice(0, 64) if h % 2 == 0 else slice(64, 128)
            bias_sb = bias_tiles.pop(h)
            exps = expsp.tile([128, 8, S], BF16, tag="exps", name=f"exps{h}")
            exps_tiles[h] = exps
            for R in range(8):
                def op(R=R):
                    trb = trbp.tile([128, 1024], BF16, tag="trb", name="trb")
                    for i in range(8):
                        nc.tensor.transpose(
                            trb[:, i * 128 : (i + 1) * 128],
                            bias_sb[:, i, R * 128 : (R + 1) * 128],
                            id_sb[:],
                        )
                    sc = scoresp.tile(
                        [128, 1024], F32, tag="scores", name="sc_ps"
                    )
                    for lc in range(2):
                        nc.tensor.matmul(
                            sc[:, lc * 512 : (lc + 1) * 512],
                            lhsT=KTb[hr, hp, R * 128 : (R + 1) * 128],
                            rhs=QTb[hr, hp, lc * 512 : (lc + 1) * 512],
                            start=True,
                            stop=True,
                        )
                    nc.vector.tensor_add(sc[:], sc[:], trb[:])
                    nc.scalar.activation(
                        exps[:, R, :],
                        sc[:],
                        EXPF,
                        bias=mb_sb[:, R : R + 1],
                    )
                yield op

        def pv_stream(h, ctxp):
            exps = exps_tiles.pop(h)
            out_sb = smallp.tile([D, 2, 512], F32, tag="out", name=f"out{h}")
            lcs = (0, 1) if h < HPC - 1 else (1, 0)
            for li, lc in enumerate(lcs):
                # the very last tile has no PSUM successor: skip the ctsb
                # staging hop and run a shorter DVE-direct chain
                last = h == HPC - 1 and li == 1
                def op(lc=lc, last=last):
                    ct_ps = ctxp.tile([128, 512], F32, tag="ctx", name="ct_ps")
                    for R in range(8):
                        nc.tensor.matmul(
                            ct_ps[0 : D + 1, :],
                            lhsT=Vb[:, R, h, :],
                            rhs=exps[:, R, lc * 512 : (lc + 1) * 512],
                            start=(R == 0),
                            stop=(R == 7),
                        )
                    den = smallp.tile([1, 512], F32, tag="den", name="den")
                    rec = smallp.tile([1, 512], F32, tag="rec", name="rec")
                    rb = smallp.tile([D, 512], F32, tag="rb", name="rb")
                    if last:
                        nc.vector.tensor_scalar_add(
                            den[:], ct_ps[D : D + 1, :], EPS
                        )
                        nc.vector.reciprocal(rec[:], den[:])
                        nc.gpsimd.partition_broadcast(rb[:], rec[:], channels=D)
                        for half in range(2):
                            hs = slice(half * 256, (half + 1) * 256)
                            nc.vector.tensor_mul(
                                out_sb[:, lc, hs], ct_ps[0:D, hs], rb[:, hs]
                            )
                            dsth = bass.AP(
                                tensor=s["out"],
                                offset=h * D * S + lc * 512 + half * 256,
                                ap=[[S, D], [1, 256]],
                            )
                            nc.sync.dma_start(dsth, out_sb[:, lc, hs])
                        return
                    else:
                        ctsb = smallp.tile(
                            [D + 1, 512], F32, tag="ctsb", name="ctsb"
                        )
                        if lc == 0:
                            nc.vector.tensor_copy(
                                out=ctsb[:], in_=ct_ps[0 : D + 1, :]
                            )
                        else:
                            nc.scalar.activation(
                                ctsb[:], ct_ps[0 : D + 1, :],
                                mybir.ActivationFunctionType.Copy,
                            )
                        nc.gpsimd.tensor_scalar_add(
                            den[:], ctsb[D : D + 1, :], EPS
                        )
                        nc.vector.reciprocal(rec[:], den[:])
                        nc.gpsimd.partition_broadcast(rb[:], rec[:], channels=D)
                        nc.gpsimd.tensor_mul(
                            out_sb[:, lc, :], ctsb[0:D, :], rb[:]
                        )
                    dst = bass.AP(
                        tensor=s["out"],
                        offset=h * D * S + lc * 512,
                        ap=[[S, D], [1, 512]],
                    )
                    nc.sync.dma_start(dst, out_sb[:, lc, :])
                yield op

        def merge(lists, weights=None):
            lists = [l for l in lists if l]
            if weights is None:
                weights = [1.0] * len(lists)
            idx = [0] * len(lists)
            total = sum(len(l) for l in lists)
            for _ in range(total):
                best, bf = None, None
                for j, l in enumerate(lists):
                    if idx[j] < len(l):
                        f = idx[j] / (len(l) * weights[j])
                        if bf is None or f < bf:
                            best, bf = j, f
                lists[best][idx[best]]()
                idx[best] += 1

        with tc.tile_pool(name="stg1b", bufs=1) as stg1b:
            hT_sb = stg1b.tile([128, HID // 128, S], BF16)
            wv_sb = stg1b.tile([128, HID // 128, ODC], BF16)
            with (
                tc.tile_pool(name="stg1a", bufs=1) as stg1a,
                tc.tile_pool(name="projp", bufs=4, space="PSUM") as projp,
            ):
                wq_sb = stg1a.tile([128, HID // 128, ODC], BF16)
                wq_r = s["wq"].ap().rearrange("(k p) o -> p k o", p=128)
                nc.sync.dma_start(wq_sb[:, 0:4, :], wq_r[:, 0:4, :])
                hT_r = s["hT"].ap().rearrange("(k p) s -> p k s", p=128)
                nc.sync.dma_start(hT_sb[:, 0:2, :], hT_r[:, 0:2, :])
                nc.sync.dma_start(wq_sb[:, 4:8, :], wq_r[:, 4:8, :])
                nc.sync.dma_start(hT_sb[:, 2:4, :], hT_r[:, 2:4, :])
                nc.sync.dma_start(hT_sb[:, 4:6, :], hT_r[:, 4:6, :])
                nc.sync.dma_start(hT_sb[:, 6:8, :], hT_r[:, 6:8, :])
                wk_sb = stg1a.tile([128, HID // 128, ODC], BF16)
                nc.sync.dma_start(
                    wk_sb[:],
                    s["wk"].ap().rearrange("(k p) o -> p k o", p=128),
                )
                nc.sync.dma_start(
                    wv_sb[:],
                    s["wv"].ap().rearrange("(k p) o -> p k o", p=128),
                )
                if s.get("load_consts"):
                    nc.sync.dma_start(s["dz_sb"][:], s["distZ"].ap())
                    nc.sync.dma_start(s["mb_sb"][:], s["mbias"].ap())
                    nc.sync.dma_start(s["id_sb"][:], s["ident"].ap())
                    nc.sync.dma_start(s["idf_sb"][:], s["identf"].ap())
                    s["load_consts"] = False
                qk_proj(wq_sb, wk_sb, hT_sb, projp)
            with tc.tile_pool(name="projvp", bufs=2, space="PSUM") as projvp:
                merge(
                    [
                        list(v_proj_stream(wv_sb, hT_sb, projvp)),
                        list(qd_stream(0)),
                        list(qd_stream(1)),
                    ],
                    weights=[1.0, 4.0, 1.6],
                )

        with (
            tc.tile_pool(name="scoresp", bufs=2, space="PSUM") as scoresp,
            tc.tile_pool(name="ctxp", bufs=1, space="PSUM") as ctxp,
        ):
            for hh in range(2, HPC + 3):
                streams = []
                weights = []
                if hh < HPC:
                    streams.append(list(qd_stream(hh)))
                    weights.append(2.5 if hh == 2 else 1.2)
                if hh < HPC + 2:
                    streams.append(list(scores_stream(hh - 2, scoresp)))
                    weights.append(0.8 if hh == 2 else (1.5 if hh >= HPC else 1.0))
                if hh >= 3:
                    streams.append(list(pv_stream(hh - 3, ctxp)))
                    weights.append(1.0)
                merge(streams, weights)


def build_program(n_reps=1):
    nc = bacc.Bacc(trn_type="TRN2", target_bir_lowering=False, debug=False)

    hT = nc.dram_tensor("hT", [HID, S], BF16, kind="ExternalInput")
    wq = nc.dram_tensor("wq", [HID, ODC], BF16, kind="ExternalInput")
    wk = nc.dram_tensor("wk", [HID, ODC], BF16, kind="ExternalInput")
    wv = nc.dram_tensor("wv", [HID, ODC], BF16, kind="ExternalInput")
    distZ = nc.dram_tensor("distZ", [128, NJ], BF16, kind="ExternalInput")
    mbias = nc.dram_tensor("mbias", [128, 8], F32, kind="ExternalInput")
    ident = nc.dram_tensor("ident", [128, 128], BF16, kind="ExternalInput")
    identf = nc.dram_tensor("identf", [128, 128], F32, kind="ExternalInput")
    # output stays transposed per head: row h*64+d holds ctx[:, h, d]
    out = nc.dram_tensor("out", [HPC * D, S], F32, kind="ExternalOutput")

    # per-head DRAM scratch for the skew-compacted qd rows
    qdr = [nc.dram_tensor(f"qdr{h}", [S * RSTRIDE], BF16) for h in range(HPC)]

    with tile.TileContext(nc) as tc:
        with tc.tile_pool(name="singles", bufs=1) as singles:
            dz_sb = singles.tile([128, NJ], BF16)
            mb_sb = singles.tile([128, 8], F32)
            id_sb = singles.tile([128, 128], BF16)
            idf_sb = singles.tile([128, 128], F32)

            QTb = singles.tile([128, 4, S], BF16)   # [od%128, od//128, s]
            KTb = singles.tile([128, 4, S], BF16)
            # V natural with ones column: [s%128, s//128, h, 65]
            Vb = singles.tile([128, 8, HPC, D + 1], BF16)

            nc.vector.memset(Vb[:, :, :, D : D + 1], 1.0)

            state = dict(
                hT=hT, wq=wq, wk=wk, wv=wv,
                distZ=distZ, mbias=mbias, ident=ident, identf=identf,
                out=out,
                dz_sb=dz_sb, mb_sb=mb_sb, id_sb=id_sb, idf_sb=idf_sb,
                QTb=QTb, KTb=KTb, Vb=Vb, qdr=qdr,
                load_consts=True,
            )
            for _rep in range(n_reps):
                state["load_consts"] = True
                _body(nc, tc, state)

    nc.compile()
    return nc


def make_core_inputs(hidden_states, attention_mask, skim_mask, Wq, Wk, Wv, dist_emb):
    """Host-side prep: returns list of 8 in_maps."""
    hidden_states = np.asarray(hidden_states, np.float32)
    attention_mask = np.asarray(attention_mask, np.float32)
    skim_mask = np.asarray(skim_mask)
    Wq = np.asarray(Wq, np.float32)
    Wk = np.asarray(Wk, np.float32)
    Wv = np.asarray(Wv, np.float32)
    dist_emb = np.asarray(dist_emb, np.float32)

    # reversed dist table: dz[d, j] = dist_emb[2047 - j, d] for j >= 1,
    # col 0 = 0; replicated in both partition halves so even heads use rows
    # 0-63 and odd heads rows 64-127.
    dz = np.zeros((128, NJ), np.float32)
    tmp = dist_emb[::-1].T  # [64, 2047]; tmp[d, i] = dist_emb[2046 - i, d]
    dz[0:64, 1:NJ] = tmp
    dz[64:128, 1:NJ] = tmp
    dz = np.ascontiguousarray(dz.astype(NPBF16))

    ident = np.ascontiguousarray(np.eye(128, dtype=NPBF16))
    identf = np.ascontiguousarray(np.eye(128, dtype=np.float32))

    in_maps = []
    for core in range(8):
        b, hh = core // 2, core % 2
        cols = slice(hh * ODC, (hh + 1) * ODC)
        hT = np.ascontiguousarray(hidden_states[b].T.astype(NPBF16))
        mb = (
            attention_mask[b, 0, 0, :] + NEG * (1.0 - skim_mask[b].astype(np.float32))
        ).astype(np.float32)
        in_maps.append(
            {
                "hT": hT,
                "wq": np.ascontiguousarray(Wq[:, cols].astype(NPBF16)),
                "wk": np.ascontiguousarray(Wk[:, cols].astype(NPBF16)),
                "wv": np.ascontiguousarray(Wv[:, cols].astype(NPBF16)),
                "distZ": dz,
                "mbias": np.ascontiguousarray(mb.reshape(8, 128).T),
                "ident": ident,
                "identf": identf,
            }
        )
    return in_maps


def kernel(
    hidden_states,
    attention_mask,
    skim_mask,
    Wq,
    bq,
    Wk,
    bk,
    Wv,
    bv,
    dist_emb,
):
    in_maps = make_core_inputs(
        hidden_states, attention_mask, skim_mask, Wq, Wk, Wv, dist_emb
    )
    nc = build_program()
    res = run_bass_kernel_spmd(nc, in_maps, core_ids=list(range(8)))
    out = np.zeros((B, S, HID), np.float32)
    for core in range(8):
        b, hh = core // 2, core % 2
        out[b, :, hh * ODC : (hh + 1) * ODC] = res.results[core]["out"].T
    return out
